# Initial kernel scaffold
#
"""SAGAN-style self-attention on 8 TRN2 NeuronCores, pure data-parallel.

Reference computation (per batch element, CH=64, H=W=64, N=4096, M=1024):
    theta = W_theta @ x          [8, N]
    phi   = pool(W_phi @ x)      [8, M]
    g     = pool(W_g @ x)        [32, M]
    E     = exp(theta^T phi)     [N, M]   (softmax numerator; no max-sub needed,
                                           scores are O(+-3))
    o     = W_o @ (g @ beta^T),  beta = E / rowsum(E)
    out   = gamma * o + x

Kernel strategy per core (2 batch elements), ~136us on silicon:
  - one fused conv matmul (W_cat = [theta|phi|g], rows at 32-aligned bases)
    per batch element, bf16 rhs from a host-provided bf16 copy of x;
    PSUM->SBUF copies on the otherwise-idle ScalarE
  - 2x2 maxpool on DVE via strided access patterns
  - fold W_o into g up front: G2T[m,:] = [ (W_o @ g)^T | 1 | 0-pad ] in the
    fp8 DoubleRow pair layout, computed directly by [32]x[128,64] matmuls
    (no transposes); the ones column makes the av-matmul emit the softmax
    denominator for free
  - scores in [m, n] layout (lhsT = phi tile, rhs = theta, bf16), exp on
    ScalarE PSUM -> fp8 SBUF; av-matmul runs fp8 DoubleRow (256-key
    contraction per matmul) accumulating [o_raw | rowsum(E)] into PSUM
    [80, n].  The ti loop is software-pipelined with skew 2 so the
    exp-dependent av matmuls never stall the PE's in-order stream.
  - normalize: denominator row DMA-reshaped [1,1024]->[64,16] so the exact
    reciprocal (~6 cyc/elem) runs on 64 lanes, DMA back, gpsimd
    partition_broadcast, one tensor_tensor mul + one fused
    scalar_tensor_tensor for gamma*o + x (bit-exact x passthrough at
    gamma=0)
"""

import os
import sys

import numpy as np

if "/opt/trn_rl_repo" not in sys.path:
    sys.path.insert(0, "/opt/trn_rl_repo")

import ml_dtypes

B, CH, H, W = 16, 64, 64, 64
N = H * W          # 4096 queries
M = N // 4         # 1024 keys (after 2x2 pool)
NCORES = 8
BPC = B // NCORES  # 2 batch elements per core

NB = 1024          # n-block width (PSUM-friendly)
NNB = N // NB      # 4 n-blocks per batch element

_BUILT = None


def _build():
    """Build + compile the per-core Bass/Tile program (cached)."""
    global _BUILT
    if _BUILT is not None:
        return _BUILT

    from contextlib import ExitStack

    import concourse.bass as bass
    import concourse.mybir as mybir
    import concourse.tile as tile
    from concourse import bacc

    f32 = mybir.dt.float32
    bf16 = mybir.dt.bfloat16
    fp8 = mybir.dt.float8e4
    DR = mybir.MatmulPerfMode.DoubleRow
    ts = bass.ts
    Exp = mybir.ActivationFunctionType.Exp
    amax = mybir.AluOpType.max
    amult = mybir.AluOpType.mult
    aadd = mybir.AluOpType.add

    nc = bacc.Bacc("TRN2", target_bir_lowering=False, debug=False)

    x_d = nc.dram_tensor("x", [BPC, 64, N], f32, kind="ExternalInput")
    xbf_d = nc.dram_tensor("xbf", [BPC, 64, N], bf16, kind="ExternalInput")
    wcat_d = nc.dram_tensor("wcat", [64, 96], bf16, kind="ExternalInput")
    wot_d = nc.dram_tensor("wot", [32, 64], bf16, kind="ExternalInput")
    gcol_d = nc.dram_tensor("gcol", [64, 1], f32, kind="ExternalInput")
    out_d = nc.dram_tensor("out", [BPC, 64, N], f32, kind="ExternalOutput")

    with tile.TileContext(nc) as tc, ExitStack() as ctx:
        pool = lambda name, bufs, **kw: ctx.enter_context(
            tc.tile_pool(name=name, bufs=bufs, **kw)
        )
        const_p = pool("const", 1)
        xb_p = pool("xb", 1)
        thp_p = pool("thp", 1)
        pgp_p = pool("pg", 1)
        g2t_p = pool("g2t", 1)
        ptmp_p = pool("ptmp", 2)
        e_p = pool("esb", 5)
        rec_p = pool("rec", 2)
        rb_p = pool("rb", 2)
        tt_p = pool("tt", 2)
        ou_p = pool("ou", 2)

        # ---- load constants + inputs (x chunks first: conv-critical) ------
        xbf = []
        for b in range(BPC):
            t = xb_p.tile([64, N], bf16, tag=f"xbf{b}", name=f"xbf{b}")
            xbf.append(t)
        for cc in range(2):
            nc.sync.dma_start(xbf[0][:, ts(cc, 512)], xbf_d[0, :, ts(cc, 512)])
        wcat_sb = const_p.tile([64, 96], bf16, tag="wcat", name="wcat")
        nc.sync.dma_start(wcat_sb[:], wcat_d[:, :])
        wot_sb = const_p.tile([32, 64], bf16, tag="wot", name="wot")
        nc.sync.dma_start(wot_sb[:], wot_d[:, :])
        gcol_sb = const_p.tile([64, 1], f32, tag="gcol", name="gcol")
        nc.sync.dma_start(gcol_sb[:], gcol_d[:, :])
        for cc in range(2, 8):
            nc.sync.dma_start(xbf[0][:, ts(cc, 512)], xbf_d[0, :, ts(cc, 512)])
        for cc in range(8):
            nc.sync.dma_start(xbf[1][:, ts(cc, 512)], xbf_d[1, :, ts(cc, 512)])
        xb = []
        for b in range(BPC):
            t = xb_p.tile([64, N], f32, tag=f"xb{b}", name=f"xb{b}")
            for cc in range(4):
                nc.sync.dma_start(t[:, ts(cc, 1024)], x_d[b, :, ts(cc, 1024)])
            xb.append(t)

        # ---- phase A: fused theta/phi/g conv, both batch elems ------------
        # psum rows: b0 -> 0..47, b1 -> 64..111 (col-tiled); [theta8|phi8|g32]
        thp = [thp_p.tile([96, N], bf16, tag=f"thp{b}", name=f"thp{b}") for b in range(BPC)]
        pa_ctx = tc.tile_pool(name="pa_ps", bufs=2, space="PSUM")
        pa_ps = pa_ctx.__enter__()
        Copy = mybir.ActivationFunctionType.Copy

        def emit_conv(b):
            for cc in range(4):  # four 1024-wide chunks
                pa_t = pa_ps.tile([96, 1024], f32, tag="pa", name="pa")
                for j in range(2):
                    nc.tensor.matmul(
                        pa_t[:, ts(j, 512)],
                        lhsT=wcat_sb[:],
                        rhs=xbf[b][:, cc * 1024 + j * 512 : cc * 1024 + (j + 1) * 512],
                        start=True,
                        stop=True,
                    )
                nc.scalar.activation(thp[b][:, ts(cc, 1024)], pa_t[:], Copy)


        # ---- phase B + G2T, per batch element ------------------------------
        # 2x2 maxpool of phi/g rows (DVE), then G2T = [(W_o @ g)^T | 1] in
        # fp8 DoubleRow pair layout: pair p covers m-tiles ti=2p, 2p+1; slot
        # cols [p*160 + i*80 + c]: c 0..63 = (W_o@g)^T, c 64 = ones
        # (denominator), c 65..79 = zero pad (DoubleRow i-step % 16 == 0).
        pg2_ctx = tc.tile_pool(name="pg2_ps", bufs=2, space="PSUM")
        pg2_ps = pg2_ctx.__enter__()
        pg_phi, pg_g, g2t = [None] * BPC, [None] * BPC, [None] * BPC

        def emit_pool(b):
            phit = pgp_p.tile([8, M], bf16, tag=f"pgphi{b}", name=f"pgphi{b}")
            gt = pgp_p.tile([32, M], bf16, tag=f"pgg{b}", name=f"pgg{b}")
            pg_phi[b], pg_g[b] = phit, gt
            for cc in range(2):
                for dst_t, lo, hi in ((phit, 32, 40), (gt, 64, 96)):
                    src = thp[b][lo:hi, ts(cc, 2048)]
                    v = src.rearrange("p (hw t) -> p hw t", t=2)
                    tmpw = pgp_p.tile([32, 1024], bf16, tag="tmpw", name="tmpw")
                    pp = hi - lo
                    nc.vector.tensor_tensor(
                        tmpw[0:pp, :], v[:, :, 0], v[:, :, 1], amax
                    )
                    v2 = tmpw[0:pp, :].rearrange("p (h t w) -> p h t w", t=2, w=32)
                    dst = dst_t[:, ts(cc, 512)].rearrange("p (h w) -> p h w", w=32)
                    nc.vector.tensor_tensor(
                        dst[:], v2[:, :, 0, :], v2[:, :, 1, :], amax
                    )

        def emit_g2t(b):
            g2 = g2t_p.tile([128, 4 * 160], fp8, tag=f"g2t{b}", name=f"g2t{b}")
            g2t[b] = g2
            nc.gpsimd.memset(g2[:], 0.0)
            g2v = g2.rearrange("p (s c) -> p s c", c=80)
            nc.gpsimd.memset(g2v[:, :, 64:65], 1.0)
            for ti in range(8):
                pg2_t = pg2_ps.tile([128, 64], f32, tag="pg2", name="pg2")
                nc.tensor.matmul(
                    pg2_t[:],
                    lhsT=pg_g[b][:, ts(ti, 128)],
                    rhs=wot_sb[:],
                    start=True,
                    stop=True,
                )
                nc.scalar.activation(g2[:, ti * 80 : ti * 80 + 64], pg2_t[:], Copy)

        emit_conv(0)
        emit_pool(0)
        emit_conv(1)
        emit_pool(1)
        emit_g2t(0)
        emit_g2t(1)
        pg2_ctx.__exit__(None, None, None)
        pa_ctx.__exit__(None, None, None)

        # ---- phase C/D: scores -> exp -> av -> normalize + residual -------
        pe_ctx = tc.tile_pool(name="pe_ps", bufs=2, space="PSUM")
        pe_ps = pe_ctx.__enter__()
        pav_ctx = tc.tile_pool(name="pav_ps", bufs=2, space="PSUM")
        pav_ps = pav_ctx.__enter__()
        for nb in range(NNB):
            for b in range(BPC):
                pav_t = pav_ps.tile([80, NB], f32, tag="pav", name="pav")
                # software-pipelined: scores(ti) issues ahead of the
                # exp-dependent av matmul so the PE in-order stream never
                # stalls behind ScalarE.  av runs fp8 DoubleRow: one matmul
                # contracts an m-tile PAIR (256 keys).
                e_pairs = {}
                for ti in range(12):
                    if ti < 8:
                        pe_t = pe_ps.tile([128, NB], f32, tag="pe", name="pe")
                        for j in range(NB // 512):
                            nc.tensor.matmul(
                                pe_t[:, ts(j, 512)],
                                lhsT=pg_phi[b][:, ts(ti, 128)],
                                rhs=thp[b][0:8, nb * NB + j * 512 : nb * NB + (j + 1) * 512],
                                start=True,
                                stop=True,
                            )
                        p = ti // 2
                        if ti % 2 == 0:
                            e_pairs[p] = e_p.tile(
                                [128, 2 * NB], fp8, tag="e", name="e"
                            )
                        nc.scalar.activation(
                            e_pairs[p][:, (ti % 2) * NB : (ti % 2) * NB + NB],
                            pe_t[:],
                            Exp,
                        )
                    if ti >= 4 and ti % 2 == 0:
                        p = ti // 2 - 2
                        e_pair = e_pairs.pop(p)
                        ev = e_pair.rearrange("q (i f) -> q i f", i=2)
                        g2v = g2t[b][:, p * 160 : (p + 1) * 160].rearrange(
                            "q (i c) -> q i c", i=2
                        )
                        for j in range(NB // 512):
                            nc.tensor.matmul(
                                pav_t[:, ts(j, 512)],
                                lhsT=g2v[:],
                                rhs=ev[:, :, j * 512 : (j + 1) * 512],
                                start=(p == 0),
                                stop=(p == 3),
                                perf_mode=DR,
                            )
                # normalize + gamma residual.  The denominator row is
                # reshaped to [64, 16] via DMA so the exact reciprocal
                # (6 cyc/elem) runs on 64 lanes instead of 1.
                drow = rec_p.tile([1, NB], f32, tag="drow", name="drow")
                nc.vector.tensor_copy(drow[:], pav_t[64:65, :])
                dsq = rec_p.tile([64, NB // 64], f32, tag="dsq", name="dsq")
                nc.sync.dma_start(dsq[:], drow[:])
                rsq = rec_p.tile([64, NB // 64], f32, tag="rsq", name="rsq")
                nc.vector.reciprocal(rsq[:], dsq[:])
                rec_t = rec_p.tile([1, NB], f32, tag="rec", name="rec")
                nc.sync.dma_start(rec_t[:], rsq[:])
                rb_t = rb_p.tile([64, NB], f32, tag="rb", name="rb")
                nc.gpsimd.partition_broadcast(rb_t[:], rec_t[0:1, :])
                t_t = tt_p.tile([64, NB], f32, tag="t", name="t")
                nc.vector.tensor_tensor(t_t[:], pav_t[0:64, :], rb_t[:], amult)
                o_t = ou_p.tile([64, NB], f32, tag="o", name="o")
                nc.vector.scalar_tensor_tensor(
                    o_t[:],
                    t_t[:],
                    gcol_sb[:, 0:1],
                    xb[b][:, ts(nb, NB)],
                    amult,
                    aadd,
                )
                nc.sync.dma_start(out_d[b, :, ts(nb, NB)], o_t[:])
        pav_ctx.__exit__(None, None, None)
        pe_ctx.__exit__(None, None, None)

    nc.compile()
    _BUILT = nc
    return nc


def _in_maps(x, W_theta, W_phi, W_g, W_o, gamma):
    x = np.asarray(x, dtype=np.float32)
    wcat = np.zeros((96, 64), dtype=np.float32)
    wcat[0:8] = np.asarray(W_theta)
    wcat[32:40] = np.asarray(W_phi)
    wcat[64:96] = np.asarray(W_g)
    wcat = np.ascontiguousarray(wcat.T).astype(ml_dtypes.bfloat16)
    wot = np.ascontiguousarray(np.asarray(W_o).T).astype(ml_dtypes.bfloat16)
    gcol = np.full((64, 1), np.float32(np.asarray(gamma)), dtype=np.float32)
    maps = []
    xbf_all = x.astype(ml_dtypes.bfloat16)
    for i in range(NCORES):
        xs = np.ascontiguousarray(x[i * BPC : (i + 1) * BPC].reshape(BPC, CH, N))
        xbfs = np.ascontiguousarray(
            xbf_all[i * BPC : (i + 1) * BPC].reshape(BPC, CH, N)
        )
        maps.append({"x": xs, "xbf": xbfs, "wcat": wcat, "wot": wot, "gcol": gcol})
    return maps


def run_shards(in_maps, **kw):
    nc = _build()
    from concourse.bass_utils import run_bass_kernel_spmd

    return run_bass_kernel_spmd(nc, in_maps, core_ids=list(range(NCORES)), **kw)


def kernel(x, W_theta, W_phi, W_g, W_o, gamma):
    res = run_shards(_in_maps(x, W_theta, W_phi, W_g, W_o, gamma))
    out = np.concatenate([res.results[i]["out"] for i in range(NCORES)], axis=0)
    return np.ascontiguousarray(out.reshape(B, CH, H, W).astype(np.float32))


if __name__ == "__main__":
    # smoke test with random data
    rng = np.random.default_rng(0)
    ins = {
        "x": rng.standard_normal((B, CH, H, W), dtype=np.float32),
        "W_theta": (rng.standard_normal((8, 64)) * 0.05).astype(np.float32),
        "W_phi": (rng.standard_normal((8, 64)) * 0.05).astype(np.float32),
        "W_g": (rng.standard_normal((32, 64)) * 0.05).astype(np.float32),
        "W_o": (rng.standard_normal((64, 32)) * 0.05).astype(np.float32),
        "gamma": np.float32(0.0),
    }
    out = kernel(**ins)
    print("out", out.shape, out.dtype, float(np.abs(out - ins["x"]).max()))



# revision 1
# speedup vs baseline: 1.0596x; 1.0596x over previous
"""SAGAN-style self-attention on 8 TRN2 NeuronCores, pure data-parallel.

Reference computation (per batch element, CH=64, H=W=64, N=4096, M=1024):
    theta = W_theta @ x          [8, N]
    phi   = pool(W_phi @ x)      [8, M]
    g     = pool(W_g @ x)        [32, M]
    E     = exp(theta^T phi)     [N, M]   (softmax numerator; no max-sub needed,
                                           scores are O(+-3))
    o     = W_o @ (g @ beta^T),  beta = E / rowsum(E)
    out   = gamma * o + x

Kernel strategy per core (2 batch elements), ~136us on silicon:
  - one fused conv matmul (W_cat = [theta|phi|g], rows at 32-aligned bases)
    per batch element, bf16 rhs from a host-provided bf16 copy of x;
    PSUM->SBUF copies on the otherwise-idle ScalarE
  - 2x2 maxpool on DVE via strided access patterns
  - fold W_o into g up front: G2T[m,:] = [ (W_o @ g)^T | 1 | 0-pad ] in the
    fp8 DoubleRow pair layout, computed directly by [32]x[128,64] matmuls
    (no transposes); the ones column makes the av-matmul emit the softmax
    denominator for free
  - scores in [m, n] layout (lhsT = phi tile, rhs = theta, bf16), exp on
    ScalarE PSUM -> fp8 SBUF; av-matmul runs fp8 DoubleRow (256-key
    contraction per matmul) accumulating [o_raw | rowsum(E)] into PSUM
    [80, n].  The ti loop is software-pipelined with skew 2 so the
    exp-dependent av matmuls never stall the PE's in-order stream.
  - normalize: denominator row DMA-reshaped [1,1024]->[64,16] so the exact
    reciprocal (~6 cyc/elem) runs on 64 lanes, DMA back, gpsimd
    partition_broadcast, one tensor_tensor mul + one fused
    scalar_tensor_tensor for gamma*o + x (bit-exact x passthrough at
    gamma=0)
"""

import os
import sys

import numpy as np

if "/opt/trn_rl_repo" not in sys.path:
    sys.path.insert(0, "/opt/trn_rl_repo")

import ml_dtypes

B, CH, H, W = 16, 64, 64, 64
N = H * W          # 4096 queries
M = N // 4         # 1024 keys (after 2x2 pool)
NCORES = 8
BPC = B // NCORES  # 2 batch elements per core

NB = 1024          # n-block width (PSUM-friendly)
NNB = N // NB      # 4 n-blocks per batch element

_BUILT = None


def _build():
    """Build + compile the per-core Bass/Tile program (cached)."""
    global _BUILT
    if _BUILT is not None:
        return _BUILT

    from contextlib import ExitStack

    import concourse.bass as bass
    import concourse.mybir as mybir
    import concourse.tile as tile
    from concourse import bacc

    f32 = mybir.dt.float32
    bf16 = mybir.dt.bfloat16
    fp8 = mybir.dt.float8e4
    DR = mybir.MatmulPerfMode.DoubleRow
    ts = bass.ts
    Exp = mybir.ActivationFunctionType.Exp
    amax = mybir.AluOpType.max
    amult = mybir.AluOpType.mult
    aadd = mybir.AluOpType.add

    nc = bacc.Bacc("TRN2", target_bir_lowering=False, debug=False)

    x_d = nc.dram_tensor("x", [BPC, 64, N], f32, kind="ExternalInput")
    xbf_d = nc.dram_tensor("xbf", [BPC, 64, N], bf16, kind="ExternalInput")
    wcat_d = nc.dram_tensor("wcat", [64, 96], bf16, kind="ExternalInput")
    wot_d = nc.dram_tensor("wot", [32, 64], bf16, kind="ExternalInput")
    gcol_d = nc.dram_tensor("gcol", [64, 1], f32, kind="ExternalInput")
    out_d = nc.dram_tensor("out", [BPC, 64, N], f32, kind="ExternalOutput")

    with tile.TileContext(nc) as tc, ExitStack() as ctx:
        pool = lambda name, bufs, **kw: ctx.enter_context(
            tc.tile_pool(name=name, bufs=bufs, **kw)
        )
        const_p = pool("const", 1)
        xb_p = pool("xb", 1)
        thp_p = pool("thp", 1)
        pgp_p = pool("pg", 1)
        g2t_p = pool("g2t", 1)
        ptmp_p = pool("ptmp", 2)
        e_p = pool("esb", 5)
        rec_p = pool("rec", 2)
        rb_p = pool("rb", 2)
        tt_p = pool("tt", 2)
        ou_p = pool("ou", 2)

        # ---- load constants + inputs (x chunks first: conv-critical) ------
        xbf = []
        for b in range(BPC):
            t = xb_p.tile([64, N], bf16, tag=f"xbf{b}", name=f"xbf{b}")
            xbf.append(t)
        for cc in range(2):
            nc.sync.dma_start(xbf[0][:, ts(cc, 512)], xbf_d[0, :, ts(cc, 512)])
        wcat_sb = const_p.tile([64, 96], bf16, tag="wcat", name="wcat")
        nc.sync.dma_start(wcat_sb[:], wcat_d[:, :])
        wot_sb = const_p.tile([32, 64], bf16, tag="wot", name="wot")
        nc.sync.dma_start(wot_sb[:], wot_d[:, :])
        gcol_sb = const_p.tile([64, 1], f32, tag="gcol", name="gcol")
        nc.sync.dma_start(gcol_sb[:], gcol_d[:, :])
        for cc in range(2, 8):
            nc.sync.dma_start(xbf[0][:, ts(cc, 512)], xbf_d[0, :, ts(cc, 512)])
        for cc in range(8):
            nc.sync.dma_start(xbf[1][:, ts(cc, 512)], xbf_d[1, :, ts(cc, 512)])
        xb = []
        for b in range(BPC):
            t = xb_p.tile([64, N], f32, tag=f"xb{b}", name=f"xb{b}")
            for cc in range(4):
                nc.sync.dma_start(t[:, ts(cc, 1024)], x_d[b, :, ts(cc, 1024)])
            xb.append(t)

        # ---- phase A: fused theta/phi/g conv, both batch elems ------------
        # psum rows: b0 -> 0..47, b1 -> 64..111 (col-tiled); [theta8|phi8|g32]
        thp = [thp_p.tile([96, N], bf16, tag=f"thp{b}", name=f"thp{b}") for b in range(BPC)]
        pa_ctx = tc.tile_pool(name="pa_ps", bufs=2, space="PSUM")
        pa_ps = pa_ctx.__enter__()
        Copy = mybir.ActivationFunctionType.Copy

        def emit_conv(b):
            for cc in range(4):  # four 1024-wide chunks
                pa_t = pa_ps.tile([96, 1024], f32, tag="pa", name="pa")
                for j in range(2):
                    nc.tensor.matmul(
                        pa_t[:, ts(j, 512)],
                        lhsT=wcat_sb[:],
                        rhs=xbf[b][:, cc * 1024 + j * 512 : cc * 1024 + (j + 1) * 512],
                        start=True,
                        stop=True,
                    )
                nc.scalar.activation(thp[b][:, ts(cc, 1024)], pa_t[:], Copy)


        # ---- phase B + G2T, per batch element ------------------------------
        # 2x2 maxpool of phi/g rows (DVE), then G2T = [(W_o @ g)^T | 1] in
        # fp8 DoubleRow pair layout: pair p covers m-tiles ti=2p, 2p+1; slot
        # cols [p*160 + i*80 + c]: c 0..63 = (W_o@g)^T, c 64 = ones
        # (denominator), c 65..79 = zero pad (DoubleRow i-step % 16 == 0).
        pg2_ctx = tc.tile_pool(name="pg2_ps", bufs=2, space="PSUM")
        pg2_ps = pg2_ctx.__enter__()
        pg_phi, pg_g, g2t = [None] * BPC, [None] * BPC, [None] * BPC

        def emit_pool(b):
            phit = pgp_p.tile([8, M], bf16, tag=f"pgphi{b}", name=f"pgphi{b}")
            gt = pgp_p.tile([32, M], bf16, tag=f"pgg{b}", name=f"pgg{b}")
            pg_phi[b], pg_g[b] = phit, gt
            for cc in range(2):
                for dst_t, lo, hi in ((phit, 32, 40), (gt, 64, 96)):
                    src = thp[b][lo:hi, ts(cc, 2048)]
                    v = src.rearrange("p (hw t) -> p hw t", t=2)
                    tmpw = pgp_p.tile([32, 1024], bf16, tag="tmpw", name="tmpw")
                    pp = hi - lo
                    nc.vector.tensor_tensor(
                        tmpw[0:pp, :], v[:, :, 0], v[:, :, 1], amax
                    )
                    v2 = tmpw[0:pp, :].rearrange("p (h t w) -> p h t w", t=2, w=32)
                    dst = dst_t[:, ts(cc, 512)].rearrange("p (h w) -> p h w", w=32)
                    nc.vector.tensor_tensor(
                        dst[:], v2[:, :, 0, :], v2[:, :, 1, :], amax
                    )

        def emit_g2t(b):
            g2 = g2t_p.tile([128, 4 * 160], fp8, tag=f"g2t{b}", name=f"g2t{b}")
            g2t[b] = g2
            nc.gpsimd.memset(g2[:], 0.0)
            g2v = g2.rearrange("p (s c) -> p s c", c=80)
            nc.gpsimd.memset(g2v[:, :, 64:65], 1.0)
            for ti in range(8):
                pg2_t = pg2_ps.tile([128, 64], f32, tag="pg2", name="pg2")
                nc.tensor.matmul(
                    pg2_t[:],
                    lhsT=pg_g[b][:, ts(ti, 128)],
                    rhs=wot_sb[:],
                    start=True,
                    stop=True,
                )
                nc.scalar.activation(g2[:, ti * 80 : ti * 80 + 64], pg2_t[:], Copy)

        emit_conv(0)
        emit_pool(0)
        emit_conv(1)
        emit_pool(1)
        emit_g2t(0)
        emit_g2t(1)
        pg2_ctx.__exit__(None, None, None)
        pa_ctx.__exit__(None, None, None)

        # ---- phase C/D: scores -> exp -> av -> normalize + residual -------
        pe_ctx = tc.tile_pool(name="pe_ps", bufs=2, space="PSUM")
        pe_ps = pe_ctx.__enter__()
        pav_ctx = tc.tile_pool(name="pav_ps", bufs=2, space="PSUM")
        pav_ps = pav_ctx.__enter__()
        for nb in range(NNB):
            for b in range(BPC):
                pav_t = pav_ps.tile([80, NB], f32, tag="pav", name="pav")
                # software-pipelined: scores(ti) issues ahead of the
                # exp-dependent av matmul so the PE in-order stream never
                # stalls behind ScalarE.  av runs fp8 DoubleRow: one matmul
                # contracts an m-tile PAIR (256 keys).
                e_pairs = {}
                for ti in range(12):
                    if ti < 8:
                        pe_t = pe_ps.tile([128, NB], f32, tag="pe", name="pe")
                        for j in range(NB // 512):
                            nc.tensor.matmul(
                                pe_t[:, ts(j, 512)],
                                lhsT=pg_phi[b][:, ts(ti, 128)],
                                rhs=thp[b][0:8, nb * NB + j * 512 : nb * NB + (j + 1) * 512],
                                start=True,
                                stop=True,
                            )
                        p = ti // 2
                        if ti % 2 == 0:
                            e_pairs[p] = e_p.tile(
                                [128, 2 * NB], fp8, tag="e", name="e"
                            )
                        nc.scalar.activation(
                            e_pairs[p][:, (ti % 2) * NB : (ti % 2) * NB + NB],
                            pe_t[:],
                            Exp,
                        )
                    if ti >= 4 and ti % 2 == 0:
                        p = ti // 2 - 2
                        e_pair = e_pairs.pop(p)
                        ev = e_pair.rearrange("q (i f) -> q i f", i=2)
                        g2v = g2t[b][:, p * 160 : (p + 1) * 160].rearrange(
                            "q (i c) -> q i c", i=2
                        )
                        for j in range(NB // 512):
                            nc.tensor.matmul(
                                pav_t[:, ts(j, 512)],
                                lhsT=g2v[:],
                                rhs=ev[:, :, j * 512 : (j + 1) * 512],
                                start=(p == 0),
                                stop=(p == 3),
                                perf_mode=DR,
                            )
                # normalize + gamma residual.  The denominator row is
                # reshaped to [64, 16] via DMA so the exact reciprocal
                # (6 cyc/elem) runs on 64 lanes instead of 1.
                drow = rec_p.tile([1, NB], f32, tag="drow", name="drow")
                nc.vector.tensor_copy(drow[:], pav_t[64:65, :])
                dsq = rec_p.tile([64, NB // 64], f32, tag="dsq", name="dsq")
                nc.sync.dma_start(dsq[:], drow[:])
                rsq = rec_p.tile([64, NB // 64], f32, tag="rsq", name="rsq")
                nc.vector.reciprocal(rsq[:], dsq[:])
                rec_t = rec_p.tile([1, NB], f32, tag="rec", name="rec")
                nc.sync.dma_start(rec_t[:], rsq[:])
                rb_t = rb_p.tile([64, NB], f32, tag="rb", name="rb")
                nc.gpsimd.partition_broadcast(rb_t[:], rec_t[0:1, :])
                t_t = tt_p.tile([64, NB], f32, tag="t", name="t")
                nc.vector.tensor_tensor(t_t[:], pav_t[0:64, :], rb_t[:], amult)
                o_t = ou_p.tile([64, NB], f32, tag="o", name="o")
                nc.vector.scalar_tensor_tensor(
                    o_t[:],
                    t_t[:],
                    gcol_sb[:, 0:1],
                    xb[b][:, ts(nb, NB)],
                    amult,
                    aadd,
                )
                nc.sync.dma_start(out_d[b, :, ts(nb, NB)], o_t[:])
        pav_ctx.__exit__(None, None, None)
        pe_ctx.__exit__(None, None, None)

    nc.compile()
    _BUILT = nc
    return nc


def _in_maps(x, W_theta, W_phi, W_g, W_o, gamma):
    x = np.asarray(x, dtype=np.float32)
    wcat = np.zeros((96, 64), dtype=np.float32)
    wcat[0:8] = np.asarray(W_theta)
    wcat[32:40] = np.asarray(W_phi)
    wcat[64:96] = np.asarray(W_g)
    wcat = np.ascontiguousarray(wcat.T).astype(ml_dtypes.bfloat16)
    wot = np.ascontiguousarray(np.asarray(W_o).T).astype(ml_dtypes.bfloat16)
    gcol = np.full((64, 1), np.float32(np.asarray(gamma)), dtype=np.float32)
    maps = []
    xbf_all = x.astype(ml_dtypes.bfloat16)
    for i in range(NCORES):
        xs = np.ascontiguousarray(x[i * BPC : (i + 1) * BPC].reshape(BPC, CH, N))
        xbfs = np.ascontiguousarray(
            xbf_all[i * BPC : (i + 1) * BPC].reshape(BPC, CH, N)
        )
        maps.append({"x": xs, "xbf": xbfs, "wcat": wcat, "wot": wot, "gcol": gcol})
    return maps


def run_shards(in_maps, **kw):
    nc = _build()
    from concourse.bass_utils import run_bass_kernel_spmd

    return run_bass_kernel_spmd(nc, in_maps, core_ids=list(range(NCORES)), **kw)


def kernel(x, W_theta, W_phi, W_g, W_o, gamma):
    res = run_shards(_in_maps(x, W_theta, W_phi, W_g, W_o, gamma))
    out = np.concatenate([res.results[i]["out"] for i in range(NCORES)], axis=0)
    return np.ascontiguousarray(out.reshape(B, CH, H, W).astype(np.float32))


if __name__ == "__main__":
    # smoke test with random data
    rng = np.random.default_rng(0)
    ins = {
        "x": rng.standard_normal((B, CH, H, W), dtype=np.float32),
        "W_theta": (rng.standard_normal((8, 64)) * 0.05).astype(np.float32),
        "W_phi": (rng.standard_normal((8, 64)) * 0.05).astype(np.float32),
        "W_g": (rng.standard_normal((32, 64)) * 0.05).astype(np.float32),
        "W_o": (rng.standard_normal((64, 32)) * 0.05).astype(np.float32),
        "gamma": np.float32(0.0),
    }
    out = kernel(**ins)
    print("out", out.shape, out.dtype, float(np.abs(out - ins["x"]).max()))

